# revision 9
# baseline (speedup 1.0000x reference)
"""KAN block (RBF-spline einsum) Trainium2 kernel, mixed bf16/fp8.

Computes out[b,o] = sum_{i,k} W[o,i,k] * exp(-0.5*((x[b,i]-knots[k])/h)^2)
for B=2048, IN=1024, OUT=1024, K=20 on 8 NeuronCores.

Strategy (v3)
-------------
Sharding: 4-way over batch x 2-way over out_features (pure-concat gather),
BC=512 batch rows and OC=512 out features per core.

Precision split over the knot axis (x ~ N(0,1), so slice k's share of the
output's quadratic weight is ~ phi(t_k)):
  - bf16 slices {7..12} (~48% of the energy): noise-free. Their g-factors
    fold into the bf16 W on the host, so on-chip they are plain
    tensor_tensor products anch10 * r^j - cheap (2x DVE mode) and legal on
    the Pool engine, which frees DVE for the fp8 work.
  - fp8 singles elsewhere (14 slices), paired into DoubleRow matmuls (4x
    the bf16 PE rate). Produced as one DVE scalar_tensor_tensor each
    ((g_aj * anchor) * r^j; g spans e^-24..e^15 so it must ride in the
    stt scalar - not foldable into fp8 W, whose denormal floor is 2^-9).
W fp8 slices are quantized with 2-tap noise shaping along k (adjacent RBFs
overlap) followed by ICM sweeps of G-weighted greedy coordinate descent
(G = analytic RBF Gram under N(0,1)), minimizing the expected output error
of the rounding. Measured rel err ~1.6e-2 vs the 2e-2 budget.

Engine balance per i-tile (ns, cost-model): PE 8108 (24 bf16 + 28 DR
matmuls), DVE ~8100 (12 stt + r2/ri2 + k9), Pool ~6700 (r3/ri3 + 4 chain
muls), ACT ~6100 (3 Square + 3 anchor Exp + rb/rib + k3/k16 fp8).
"""

import math
import sys

import numpy as np

for _p in ("/opt/trn_rl_repo",):
    if _p not in sys.path:
        sys.path.append(_p)

import ml_dtypes

import concourse.bass as bass
import concourse.tile as tile
from concourse import bacc, mybir
from concourse import bass_utils

F32 = mybir.dt.float32
BF16 = mybir.dt.bfloat16
F8E4 = mybir.dt.float8e4
AF = mybir.ActivationFunctionType
PM = mybir.MatmulPerfMode
ALU = mybir.AluOpType

B, IN, OUT, K = 2048, 1024, 1024, 20
N_CORES = 8
B_SHARDS, O_SHARDS = 4, 2
BC = B // B_SHARDS          # 512 batch rows per core
OC = OUT // O_SHARDS        # 512 out features per core
N_ITILES = IN // 128        # 8
N_OTILES = OC // 128        # 4

# ---- precision split over k --------------------------------------------
ANCH = (3, 10, 16)
# bf16 slices in bbf/wt slot order (k10 = anch10 itself is slot 0);
# ordered by production time so the PE can interleave ready work
BF16_SET = (10, 9, 11, 8, 12, 7)
# fp8 slot order; adjacent slots form the DoubleRow pairs, grouped by
# production wave: k3/k16/k0 from ACT, then j=+-1 slices (need only
# rb/rib), j=+-2 (r2/ri2), j=+-3 (r3/ri3)
F8_SLOTS = (3, 16, 2, 4, 15, 17, 1, 5, 14, 18, 0, 6, 13, 19)
ACT_DIRECT = (0,)                  # straggler slice moved to ACT Square+Exp
NBF = len(BF16_SET)                # 6
NS8 = len(F8_SLOTS)                # 14
N_DR = NS8 // 2                    # 7
NS_A, NS_B = 0.9, -0.2             # W noise-shaping filter
ICM_SWEEPS = 2
WARMUP_MM = 17

_cache: dict = {}


def _a_of(k):
    return min(ANCH, key=lambda a: abs(k - a))


def _build_program(h: float, t0: float, reps: int = 1, unroll: int = 4):
    """Build + compile the single-core Bass program (same for all cores)."""
    nc = bacc.Bacc(
        "TRN2",
        target_bir_lowering=False,
        debug=False,
        enable_asserts=False,
        num_devices=N_CORES,
    )
    xt_d = nc.dram_tensor("xt", [IN, BC], F32, kind="ExternalInput")
    wt_d = nc.dram_tensor("wt", [N_ITILES, 128, NBF, OC], BF16, kind="ExternalInput")
    w8_d = nc.dram_tensor("w8", [N_ITILES, 128, NS8, OC], F8E4, kind="ExternalInput")
    out_d = nc.dram_tensor("out", [OC, BC], F32, kind="ExternalOutput")
    xt, wt, w8, out = xt_d.ap(), wt_d.ap(), w8_d.ap(), out_d.ap()

    inv_h = 1.0 / h
    s2h = 1.0 / (math.sqrt(2.0) * h)
    knots = [t0 + k * h for k in range(K)]

    def gfac(a, j):
        return math.exp(-j * knots[a] * inv_h - 0.5 * j * j)

    slot = {k: i for i, k in enumerate(F8_SLOTS)}
    bslot = {k: i for i, k in enumerate(BF16_SET)}

    with tile.TileContext(nc) as tc:
        with (
            tc.tile_pool(name="xp", bufs=2) as xp,
            tc.tile_pool(name="wp", bufs=3) as wp,
            tc.tile_pool(name="w8p", bufs=3) as w8p,
            tc.tile_pool(name="rp", bufs=2) as rp,
            tc.tile_pool(name="sp", bufs=2) as sp,
            tc.tile_pool(name="ap_", bufs=2) as apool,
            tc.tile_pool(name="bp", bufs=3) as bp,
            tc.tile_pool(name="b8p", bufs=3) as b8p,
            tc.tile_pool(name="op", bufs=1) as op,
            tc.tile_pool(name="cp", bufs=1) as cp,
            tc.tile_pool(name="ps", bufs=2, space=bass.MemorySpace.PSUM) as ps,
        ):
            # bias constants for the ACT Square ops (const_aps has no
            # registered pool under Bacc, so build them as memset tiles)
            sq_bias = {}
            for a in ANCH + ACT_DIRECT:
                bt = cp.tile([128, 1], F32, tag=f"sb{a}", name=f"sb{a}")
                nc.gpsimd.memset(bt[:], -knots[a] * s2h)
                sq_bias[a] = bt
            # dummy activation: pulls the exp table load into the constant
            # setup region so it overlaps the first input DMAs
            warm = cp.tile([128, 1], F32, tag="warm", name="warm")
            nc.gpsimd.memset(warm[:], 0.0)
            nc.scalar.activation(warm[:], warm[:], AF.Exp, scale=0.0)
            # dummy matmul chain trips the HAM clock gate to full p-state
            # during the initial DMA fill
            if WARMUP_MM:
                wsc = cp.tile([128, 512], BF16, tag="wsc", name="wsc")
                nc.gpsimd.memset(wsc[:], 1.0)
                ps_w = ps.tile([128, 512], F32, tag="ps0", name="psw")
                for _w in range(WARMUP_MM):
                    nc.tensor.matmul(ps_w[:], wsc[:, 0:128], wsc[:],
                                     start=True, stop=True)

            def body(_=None):
                psum = [
                    ps.tile([128, BC], F32, tag=f"ps{u}", name=f"ps{u}")
                    for u in range(N_OTILES)
                ]
                x_tiles = {}
                x0 = xp.tile([128, BC], F32, tag="x", name="x_t0")
                nc.sync.dma_start(x0[:], xt[0:128, :])
                x_tiles[0] = x0
                for it in range(N_ITILES):
                    x_t = x_tiles.pop(it)
                    if it + 1 < N_ITILES:
                        xn = xp.tile([128, BC], F32, tag="x", name="x_tn")
                        nc.sync.dma_start(
                            xn[:], xt[(it + 1) * 128:(it + 2) * 128, :]
                        )
                        x_tiles[it + 1] = xn
                    w_t = wp.tile([128, NBF, OC], BF16, tag="w", name="w_t")
                    nc.sync.dma_start(w_t[:], wt[it, :, :, :])
                    w8_t = w8p.tile([128, NS8, OC], F8E4, tag="w8", name="w8_t")
                    nc.sync.dma_start(w8_t[:], w8[it, :, :, :])

                    bbf = bp.tile([128, NBF, BC], BF16, tag="bbf", name="bbf")
                    b8 = b8p.tile([128, NS8, BC], F8E4, tag="b8", name="b8")

                    # --- ACT: anch10 + powers first (they feed everything),
                    # then edge anchors, then the ACT-written fp8 slices
                    sqs = {}

                    def mk_sq(a):
                        sq = sp.tile([128, BC], F32, tag=f"sq{a}", name=f"sq{a}")
                        nc.scalar.activation(
                            sq[:], x_t[:], AF.Square, scale=s2h,
                            bias=sq_bias[a][:],
                        )
                        sqs[a] = sq

                    mk_sq(10)
                    # anch10 doubles as the k10 bf16 slice (bbf slot 0)
                    nc.scalar.activation(
                        bbf[:, bslot[10], :], sqs[10][:], AF.Exp, scale=-1.0
                    )
                    anch = {10: bbf[:, bslot[10], :]}
                    rb = rp.tile([128, BC], BF16, tag="rb", name="rb")
                    rib = rp.tile([128, BC], BF16, tag="rib", name="rib")
                    nc.scalar.activation(rb[:], x_t[:], AF.Exp, scale=inv_h)
                    nc.scalar.activation(rib[:], x_t[:], AF.Exp, scale=-inv_h)
                    for a in (3, 16):
                        mk_sq(a)
                        ea = apool.tile([128, BC], BF16, tag=f"ea{a}",
                                        name=f"ea{a}")
                        nc.scalar.activation(ea[:], sqs[a][:], AF.Exp,
                                             scale=-1.0)
                        anch[a] = ea[:]
                    # ACT-written fp8 slices: the anchors + straggler k0
                    nc.scalar.activation(
                        b8[:, slot[3], :], sqs[3][:], AF.Exp, scale=-1.0
                    )
                    nc.scalar.activation(
                        b8[:, slot[16], :], sqs[16][:], AF.Exp, scale=-1.0
                    )
                    for k in ACT_DIRECT:
                        mk_sq(k)
                        nc.scalar.activation(
                            b8[:, slot[k], :], sqs[k][:], AF.Exp, scale=-1.0
                        )

                    # --- power ladder (bf16): r2/ri2 on DVE, r3/ri3 on Pool
                    r2b = rp.tile([128, BC], BF16, tag="r2b", name="r2b")
                    ri2b = rp.tile([128, BC], BF16, tag="ri2b", name="ri2b")
                    r3b = rp.tile([128, BC], BF16, tag="r3b", name="r3b")
                    ri3b = rp.tile([128, BC], BF16, tag="ri3b", name="ri3b")
                    nc.vector.tensor_mul(r2b[:], rb[:], rb[:])
                    nc.vector.tensor_mul(ri2b[:], rib[:], rib[:])
                    pw = {1: rb, -1: rib, 2: r2b, -2: ri2b, 3: r3b, -3: ri3b}

                    # --- fp8 offset singles: DVE stt (g * anch) * r^j,
                    # issued in dependency waves (j=+-1, +-2, +-3)
                    def stt(k):
                        a = _a_of(k)
                        j = k - a
                        nc.vector.scalar_tensor_tensor(
                            b8[:, slot[k], :], anch[a], gfac(a, j), pw[j][:],
                            ALU.mult, ALU.mult,
                        )

                    # Pool: chain slices + r3/ri3 ladder, interleaved with
                    # DVE waves so both engines start as inputs land
                    nc.gpsimd.tensor_mul(
                        bbf[:, bslot[11], :], anch[10], rb[:]
                    )
                    nc.vector.tensor_mul(
                        bbf[:, bslot[9], :], anch[10], rib[:]
                    )
                    for k in (2, 4, 15, 17):      # wave 1: rb/rib only
                        stt(k)
                    nc.gpsimd.tensor_mul(r3b[:], r2b[:], rb[:])
                    nc.gpsimd.tensor_mul(ri3b[:], ri2b[:], rib[:])
                    nc.gpsimd.tensor_mul(
                        bbf[:, bslot[8], :], anch[10], ri2b[:]
                    )
                    nc.gpsimd.tensor_mul(
                        bbf[:, bslot[12], :], anch[10], r2b[:]
                    )
                    for k in (1, 5, 14, 18):      # wave 2: r2/ri2
                        stt(k)
                    nc.gpsimd.tensor_mul(
                        bbf[:, bslot[7], :], anch[10], ri3b[:]
                    )
                    for k in (6, 13, 19):         # wave 3: r3/ri3
                        stt(k)

                    # --- PE: matmuls interleaved in production order
                    def mm_bf(bi, ot, start=False):
                        nc.tensor.matmul(
                            psum[ot][:],
                            w_t[:, bi, ot * 128:(ot + 1) * 128],
                            bbf[:, bi, :],
                            start=start, stop=False,
                        )

                    def mm_dr(p, ot, stop=False):
                        nc.tensor.matmul(
                            psum[ot][:],
                            w8_t[:, 2 * p:2 * p + 2, ot * 128:(ot + 1) * 128],
                            b8[:, 2 * p:2 * p + 2, :],
                            start=False, stop=stop,
                            perf_mode=PM.DoubleRow,
                        )

                    # (kind, idx) in rough production order; DR pair p covers
                    # F8_SLOTS[2p:2p+2]
                    mm_order = [("bf", 0), ("bf", 1), ("bf", 2), ("dr", 0),
                                ("bf", 3), ("dr", 1), ("dr", 2), ("bf", 4),
                                ("dr", 3), ("dr", 4), ("bf", 5), ("dr", 5),
                                ("dr", 6)]
                    last = it == N_ITILES - 1
                    if not last:
                        for mi, (kind, idx) in enumerate(mm_order):
                            for ot in range(N_OTILES):
                                if kind == "bf":
                                    mm_bf(idx, ot, start=(it == 0 and mi == 0))
                                else:
                                    mm_dr(idx, ot)
                    else:
                        # finish one psum bank at a time so copy-out + DMA
                        # overlap the remaining matmuls
                        for ot in range(N_OTILES):
                            for kind, idx in mm_order:
                                if kind == "bf":
                                    mm_bf(idx, ot)
                                else:
                                    mm_dr(idx, ot, stop=(idx == N_DR - 1))
                            o_t = op.tile([128, BC], F32, tag=f"o{ot}",
                                          name=f"o_t{ot}")
                            if ot % 2 == 0:
                                nc.vector.tensor_copy(o_t[:], psum[ot][:])
                            else:
                                nc.scalar.copy(o_t[:], psum[ot][:])
                            nc.sync.dma_start(
                                out[ot * 128:(ot + 1) * 128, :], o_t[:]
                            )

            if reps == 1:
                body()
            else:
                u = unroll if reps % unroll == 0 else 1
                with tc.For_i(0, reps // u, 1) as _i:
                    for _ in range(u):
                        body(_i)

    nc.compile()
    return nc


def _get_program(h: float, t0: float, reps: int = 1):
    key = (round(h, 9), round(t0, 9), reps)
    if key not in _cache:
        _cache[key] = _build_program(h, t0, reps)
    return _cache[key]


# ---------------------------------------------------------------------------
# Host-side W quantization
# ---------------------------------------------------------------------------

def _gram(knots, h):
    """Analytic Gram G_jk = E_{x~N(0,1)}[basis_j(x) basis_k(x)]."""
    t = np.asarray(knots, dtype=np.float64)
    m = 0.5 * (t[:, None] + t[None, :])
    d = t[:, None] - t[None, :]
    return (np.exp(-d * d / (4 * h * h))
            * h / math.sqrt(h * h + 2.0)
            * np.exp(-m * m / (2.0 + h * h)))


def _ulp8(v):
    av = np.abs(v)
    step = np.where(av > 0,
                    2.0 ** (np.floor(np.log2(np.maximum(av, 1e-300))) - 3),
                    2.0 ** -9)
    return np.maximum(step, 2.0 ** -9)


def _q8(a):
    return np.asarray(a, dtype=np.float32).astype(
        ml_dtypes.float8_e4m3).astype(np.float64)


def _quantize_W(W, knots, h):
    """Per-slice quantization: bf16 slices g-folded bf16; fp8 singles
    noise-shaped + ICM-polished against the analytic Gram."""
    G = _gram(knots, h)
    inv_h = 1.0 / h

    def gfac(a, j):
        return math.exp(-j * knots[a] * inv_h - 0.5 * j * j)

    Wf = W.astype(np.float64)
    fp8_ks = [k for k in range(K) if k not in BF16_SET]

    # --- pass 1: noise shaping along k over the fp8 slices; pending
    # residual absorbed by the bf16 block when crossed
    q = {}
    e1 = np.zeros((OUT, IN)); e2 = np.zeros((OUT, IN))
    bf_adj = {k: np.zeros((OUT, IN)) for k in BF16_SET}
    for k in range(K):
        if k in BF16_SET:
            bf_adj[k] = NS_A * e1 + NS_B * e2
            e1 = np.zeros((OUT, IN)); e2 = np.zeros((OUT, IN))
            continue
        v = Wf[:, :, k] + NS_A * e1 + NS_B * e2
        qq = _q8(v)
        q[k] = qq
        e2 = e1; e1 = v - qq

    # --- pass 2: ICM sweeps on the fp8 slices (G-weighted), bf16 fixed
    Gs = G[np.ix_(fp8_ks, fp8_ks)]
    tgt = np.stack([Wf[:, :, k] for k in fp8_ks], axis=-1).reshape(
        -1, len(fp8_ks))
    cur = np.stack([q[k] for k in fp8_ks], axis=-1).reshape(-1, len(fp8_ks))
    e = cur - tgt
    Ge = e @ Gs
    for _s in range(ICM_SWEEPS):
        for ki in range(len(fp8_ks)):
            v = tgt[:, ki]
            base = v + e[:, ki]
            step = _ulp8(base)
            lo = np.floor(base / step) * step
            rest = Ge[:, ki] - Gs[ki, ki] * e[:, ki]
            best_c = None
            best_f = None
            for c in (lo - step, lo, lo + step, lo + 2 * step):
                ce = _q8(c) - v
                f = Gs[ki, ki] * ce * ce + 2 * ce * rest
                if best_f is None:
                    best_f, best_c = f, ce
                else:
                    m = f < best_f
                    best_f = np.where(m, f, best_f)
                    best_c = np.where(m, ce, best_c)
            delta = best_c - e[:, ki]
            Ge += delta[:, None] * Gs[ki][None, :]
            e[:, ki] = best_c
    cur = (tgt + e).reshape(OUT, IN, len(fp8_ks))

    out = {}
    for ki, k in enumerate(fp8_ks):
        out[k] = _q8(cur[:, :, ki])
    for k in BF16_SET:
        a = _a_of(k)
        out[k] = (Wf[:, :, k] + bf_adj[k]) * gfac(a, k - a)
    return out


def _prep_inputs(x, W, knots):
    """Host-side sharding/layout. Returns in_maps for the 8 cores."""
    x = np.asarray(x, dtype=np.float32)
    W = np.asarray(W, dtype=np.float32)
    knots = np.asarray(knots, dtype=np.float64)
    h = float(knots[1] - knots[0])
    t0 = float(knots[0])

    Wq = _quantize_W(W, knots, h)

    wts, w8s = [], []
    for os_ in range(O_SHARDS):
        sl = slice(os_ * OC, (os_ + 1) * OC)
        wb = np.stack([Wq[k][sl] for k in BF16_SET], axis=-1)  # (OC, IN, NBF)
        wt = np.ascontiguousarray(
            wb.transpose(1, 2, 0).reshape(N_ITILES, 128, NBF, OC)
        )
        wts.append(wt.astype(ml_dtypes.bfloat16))
        we = np.stack([Wq[k][sl] for k in F8_SLOTS], axis=-1)  # (OC, IN, NS8)
        w8 = np.ascontiguousarray(
            we.transpose(1, 2, 0).reshape(N_ITILES, 128, NS8, OC)
        )
        w8s.append(w8.astype(ml_dtypes.float8_e4m3))
    xts = []
    for bs in range(B_SHARDS):
        xts.append(np.ascontiguousarray(x[bs * BC:(bs + 1) * BC].T))  # (IN, BC)

    in_maps = []
    for c in range(N_CORES):
        bs, os_ = divmod(c, O_SHARDS)
        in_maps.append({"xt": xts[bs], "wt": wts[os_], "w8": w8s[os_]})
    return in_maps, h, t0


def kernel(x, W, knots):
    assert x.shape == (B, IN) and W.shape == (OUT, IN, K) and knots.shape == (K,)
    in_maps, h, t0 = _prep_inputs(x, W, knots)
    nc = _get_program(h, t0, reps=1)
    res = bass_utils.run_bass_kernel_spmd(nc, in_maps, core_ids=list(range(N_CORES)))
    out = np.empty((B, OUT), dtype=np.float32)
    for c in range(N_CORES):
        bs, os_ = divmod(c, O_SHARDS)
        shard = res.results[c]["out"]  # (OC, BC) [o, b]
        out[bs * BC:(bs + 1) * BC, os_ * OC:(os_ + 1) * OC] = shard.T
    return out
